# revision 29
# baseline (speedup 1.0000x reference)
"""Trainium2 Bass kernel for SAGAN-style spatial self-attention.

Reference computation (per batch b):
    xf = x[b].reshape(C, N)                    # C=256, N=64*64=4096
    f  = w1 @ xf                               # [32, N]   (query^T)
    g  = w2 @ xf                               # [32, N]   (key)
    V  = (w3 @ xf)^T                           # [N, C]    (value)
    S  = f^T @ g                               # [N, N]
    O  = softmax(S, axis=-1) @ V               # [N, C]
    out[b] = O^T.reshape(C, H, W) + x[b]

Sharding: 8 cores = 4 batches x 2 query-halves. Each core holds its batch's
full xf (for keys/values) and computes attention for 2048 query positions.
No cross-core communication.

Host-side, each core's key axis is permuted so its own 2048 query columns
come first: the f-projection then reads the same SBUF tile as g/V (no
separate xq upload), and softmax/PV are key-permutation invariant.

Per-core device algorithm (n = this core's 2048 query cols, m = all 4096 keys):
  - projections f [32,2048], g [32,4096] in fp16; V [4096,257] in bf16
    (column 256 of V is ones -> PV matmul emits softmax denominator for free)
  - S^T chunks: matmul(lhsT=g_mtile [32,128], rhs=f_chunk [32,512]) -> PSUM
  - P^T = exp(S^T) -> SBUF bf16 (no max subtraction: |S| <~ 45, exp fits fp32)
  - O chunk: matmul(lhsT=P^T [128m,128n], rhs=V [128m,257]) accumulated over
    32 m-tiles -> [128n, 257]; r = 1/col256; O *= r
  - residual add in [n, C] layout, fp16 output in a [128, J, C] DRAM layout
    (16KB contiguous per partition row -> full-rate output DMA packets);
    the [C, n] transpose happens in the host-side gather.

fp16 (not fp32/fp32r) operands everywhere on the PE: fp32-mode matmuls do not
register as PE-busy for the HAM clock gate and the PE gets stuck at 1.2GHz.
fp16 keeps full clock and has enough mantissa (2^-11) for the pre-exp scores.

DMA: few, large descriptors (descriptor issue costs ~0.7us each on the
issuing queue). xkv on the sync queue (8 x 0.5MB), weights + residual on the
scalar queue. S^T groups are emitted in adjacent pairs so the PE pays the
PV<->S^T transition penalty half as often.
"""

import sys

sys.path.insert(0, "/opt/trn_rl_repo")

from contextlib import ExitStack

import numpy as np

import concourse.bass as bass
import concourse.tile as tile
from concourse import bacc, mybir
from concourse.bass import ts, ds
from concourse.bass_utils import run_bass_kernel_spmd

F32 = mybir.dt.float32
F16 = mybir.dt.float16
BF16 = mybir.dt.bfloat16

B, C, H, W = 4, 256, 64, 64
N = H * W          # 4096 keys per batch
NQ = N // 2        # 2048 queries per core
CK = 32            # query/key head dim
MT = N // 128      # 32 m-tiles
NCHUNK = NQ // 512  # 4 n-chunks of 512 query cols
NJ = NQ // 128     # 16 output row-tiles
N_WARM = 10        # HAM warmup matmuls (PE clock ramp) while input DMAs land
EXP = mybir.ActivationFunctionType.Exp


def build_nc():
    nc = bacc.Bacc("TRN2", target_bir_lowering=False, debug=False, num_devices=8)
    xkv_d = nc.dram_tensor("xkv", [C, N], F16, kind="ExternalInput")
    xqt_d = nc.dram_tensor("xqt", [128, NJ, C], F16, kind="ExternalInput")
    w1t_d = nc.dram_tensor("w1t", [C, CK], F16, kind="ExternalInput")
    w2t_d = nc.dram_tensor("w2t", [C, CK], F16, kind="ExternalInput")
    w3t_d = nc.dram_tensor("w3t", [C, C], F16, kind="ExternalInput")
    out_d = nc.dram_tensor("out", [128, NJ, C], F16, kind="ExternalOutput")

    with tile.TileContext(nc) as tc, ExitStack() as ctx:
        _body(ctx, tc, xkv_d.ap(), xqt_d.ap(), w1t_d.ap(), w2t_d.ap(),
              w3t_d.ap(), out_d.ap())
    nc.compile()
    return nc


def _body(ctx, tc, xkv_d, xqt_d, w1t_d, w2t_d, w3t_d, out_d):
    nc = tc.nc
    singles = ctx.enter_context(tc.tile_pool(name="singles", bufs=1))

    xqt = singles.tile([128, NJ, C], F16, tag="xqt", name="xqt")
    xkv_h = singles.tile([128, 2, N], F16, tag="xkv_h", name="xkv_h")
    w1t = singles.tile([128, 2, CK], F16, tag="w1t", name="w1t")
    w2t = singles.tile([128, 2, CK], F16, tag="w2t", name="w2t")
    w3t = singles.tile([128, 2, C], F16, tag="w3t", name="w3t")
    g_sb = singles.tile([CK, N], F16, tag="g_sb", name="g_sb")
    f_sb = singles.tile([CK, NQ], F16, tag="f_sb", name="f_sb")
    V = singles.tile([128, MT, 260], BF16, tag="V", name="V")
    warm = singles.tile([128, 512], BF16, tag="warm", name="warm")

    nc.vector.memset(V[:, :, 256:257], 1.0)
    nc.vector.memset(warm[:], 0.0)

    # PSUM: the S^T pool (2-bank slots, bufs=2) + a 1-bank pool (bufs=4) for
    # the PV accumulators and all projection outputs. 4 + 4 = 8 banks.
    stp = ctx.enter_context(tc.tile_pool(name="st_ps", bufs=2, space="PSUM"))
    op = ctx.enter_context(tc.tile_pool(name="o_ps", bufs=4, space="PSUM"))
    ptp = ctx.enter_context(tc.tile_pool(name="pt", bufs=2))
    stgp = ctx.enter_context(tc.tile_pool(name="stage", bufs=3))

    Pt = [None, None]
    stage = [None, None]
    posts = []

    def emit_post(item):
        cc, j, o_ps, stg = item
        J = cc * 4 + j
        r = stgp.tile([128, 1], F32, tag="r", name="r")
        if cc < NCHUNK - 1:
            nc.vector.reciprocal(r[:], o_ps[:, 256:257])
            nc.vector.tensor_scalar_mul(stg[:, j, :], o_ps[:, 0:256], r[:])
            nc.vector.tensor_add(stg[:, j, :], stg[:, j, :], xqt[:, J, :])
            if j == 3:
                nc.sync.dma_start(out_d[:, 4 * cc:4 * cc + 4, :], stg[:, :, :])
        elif j < 3:
            # final chunk: ACT is idle (no exps left) — do the normalize
            # there and ship each row-tile as soon as it is ready
            nc.vector.reciprocal(r[:], o_ps[:, 256:257])
            nc.scalar.mul(stg[:, j, :], o_ps[:, 0:256], r[:])
            nc.vector.tensor_add(stg[:, j, :], stg[:, j, :], xqt[:, J, :])
            eng = (nc.sync, nc.scalar, nc.sync)[j]
            eng.dma_start(out_d[:, 4 * cc + j, :], stg[:, j, :])
        else:
            # last row-tile: partition-split pipeline across DVE/ACT and
            # two DMA queues so the post-matmul drain stays under ~2us
            nc.vector.reciprocal(r[:], o_ps[:, 256:257])
            Jf = 4 * cc + 3
            for h, eng in ((0, nc.scalar), (1, nc.sync)):
                hp = ds(64 * h, 64)
                nc.scalar.mul(stg[hp, j, :], o_ps[hp, 0:256], r[hp, :])
                nc.vector.tensor_add(stg[hp, j, :], stg[hp, j, :],
                                     xqt[hp, J, :])
                eng.dma_start(out_d[hp, Jf, :], stg[hp, 3, :])

    def st_mm(st, c, gidx, t):
        mt = 2 * gidx + t
        nc.tensor.matmul(st[:, t, :], g_sb[:, ts(mt, 128)],
                         f_sb[:, ts(c, 512)], start=True, stop=True)

    def st_group(c, gidx):
        st = stp.tile([128, 2, 512], F32, tag="st", name="st")
        st_mm(st, c, gidx, 0)
        st_mm(st, c, gidx, 1)
        nc.scalar.activation(Pt[c % 2][:, 2 * gidx:2 * gidx + 2, :], st[:], EXP)

    # HAM warmup: the PE clock-gate opens only after ~3.4us of gapless
    # streaming; run a dummy dense bf16 burst while the input DMAs land so
    # the projection phase starts near 2.4GHz instead of 0.65.
    wps = stp.tile([128, 2, 512], F32, tag="st", name="wps")
    for i in range(N_WARM):
        nc.tensor.matmul(wps[:, i % 2, :], warm[:, 0:128], warm[:],
                         start=True, stop=True)

    # ---- input DMAs (fp16 operands are cast host-side) ----
    # scalar queue: weights then residual; sync queue: xkv in key order
    # (query-half columns first, so the f-projection can start earliest)
    for k in range(2):
        nc.scalar.dma_start(w1t[:, k, :], w1t_d[ts(k, 128), :])
    for k in range(2):
        nc.scalar.dma_start(w2t[:, k, :], w2t_d[ts(k, 128), :])
        nc.scalar.dma_start(w3t[:, k, :], w3t_d[ts(k, 128), :])
    nc.scalar.dma_start(xqt[:], xqt_d[:])
    for piece in range(4):
        for k in range(2):
            nc.sync.dma_start(xkv_h[:, k, ts(piece, 1024)],
                              xkv_d[ts(k, 128), ts(piece, 1024)])

    # ---- f chunk 0 leads (queries = first 4 key chunks), so the S^T pair 0
    # -> ACT exp chain (the critical path to the PV phase start) begins as
    # early as possible; the f tail is folded into the first loop
    # iterations. ----
    Pt[0] = ptp.tile([128, MT, 512], BF16, tag="pt", name="pt")

    def f_proj(ch):
        fp = op.tile([CK, 512], F32, tag="o", name="fp")
        for k in range(2):
            nc.tensor.matmul(fp[:], w1t[:, k, :], xkv_h[:, k, ts(ch, 512)],
                             start=(k == 0), stop=(k == 1))
        nc.vector.tensor_copy(f_sb[:, ts(ch, 512)], fp[:])

    f_proj(0)
    # g-projection, V-projection, and chunk-0 scores interleaved in one cycle
    # per 512-col chunk; S^T lags g by one chunk so the PE never waits on the
    # DVE g-copy. The V tiles keep the PE dense while ACT drains the exps.
    for ch in range(N // 512):
        if ch >= 1:
            st_group(0, 2 * (ch - 1))
            st_group(0, 2 * ch - 1)
        gp = op.tile([CK, 512], F32, tag="o", name="gp")
        for k in range(2):
            nc.tensor.matmul(gp[:], w2t[:, k, :], xkv_h[:, k, ts(ch, 512)],
                             start=(k == 0), stop=(k == 1))
        nc.vector.tensor_copy(g_sb[:, ts(ch, 512)], gp[:])
        if ch < 3:
            f_proj(ch + 1)
        for mt in range(4 * ch, 4 * ch + 4):
            vp = op.tile([128, 256], F32, tag="o", name="vp")
            for k in range(2):
                nc.tensor.matmul(vp[:], xkv_h[:, k, ts(mt, 128)], w3t[:, k, :],
                                 start=(k == 0), stop=(k == 1))
            nc.vector.tensor_copy(V[:, mt, 0:256], vp[:])
    st_group(0, 14)
    st_group(0, 15)

    # ---- attention chunks 1..NCHUNK, software-pipelined by one chunk ----
    for c in range(1, NCHUNK + 1):
        if c < NCHUNK:
            Pt[c % 2] = ptp.tile([128, MT, 512], BF16, tag="pt", name="pt")
        stage[(c - 1) % 2] = stgp.tile([128, 4, 256], F16, tag="stage", name="stage")
        o_cur = None
        for gidx in range(16):
            j, seg = gidx // 4, gidx % 4
            if seg == 0:
                o_cur = op.tile([128, 257], F32, tag="o", name="o")
            for mm in range(4):
                mt = seg * 8 + mm
                nc.tensor.matmul(o_cur[:], Pt[(c - 1) % 2][:, mt, ts(j, 128)],
                                 V[:, mt, 0:257],
                                 start=(mt == 0), stop=(mt == MT - 1),
                                 skip_group_check=True)
            # S^T groups in adjacent pairs: half as many PV<->S^T stream
            # transitions on the PE (each costs ~100ns of drained pipeline).
            # Odd slots: each chunk's first pair then trails the previous
            # chunk's exp backlog on ACT instead of stalling against it.
            if c < NCHUNK and gidx % 2 == 1:
                st_group(c, gidx - 1)
                st_group(c, gidx)
            for mm in range(4, 8):
                mt = seg * 8 + mm
                nc.tensor.matmul(o_cur[:], Pt[(c - 1) % 2][:, mt, ts(j, 128)],
                                 V[:, mt, 0:257],
                                 start=(mt == 0), stop=(mt == MT - 1),
                                 skip_group_check=True)
            if seg == 3:
                posts.append((c - 1, j, o_cur, stage[(c - 1) % 2]))
            # delay each n-tile's post-processing by one PE group so the DVE
            # normalize never stalls the PE stream; the final chunk has no
            # S^T stream left to protect, so flush immediately there
            while len(posts) > 0:
                emit_post(posts.pop(0))
    while posts:
        emit_post(posts.pop(0))


_NC_CACHE = None


def _get_nc():
    global _NC_CACHE
    if _NC_CACHE is None:
        _NC_CACHE = build_nc()
    return _NC_CACHE


def make_in_maps(x, w1, w2, w3):
    x = np.ascontiguousarray(x, dtype=np.float32).reshape(B, C, N)
    w1t = np.ascontiguousarray(w1.T, dtype=np.float16)
    w2t = np.ascontiguousarray(w2.T, dtype=np.float16)
    w3t = np.ascontiguousarray(w3.T, dtype=np.float16)
    in_maps = []
    xh = x.astype(np.float16)
    for core in range(8):
        b, half = core // 2, core % 2
        qsl = slice(half * NQ, (half + 1) * NQ)
        osl = slice((1 - half) * NQ, (2 - half) * NQ)
        # key axis permuted: own query columns first
        xkv = np.concatenate([xh[b][:, qsl], xh[b][:, osl]], axis=1)
        # residual for this core's queries: [NQ, C] -> [128, NJ, C]
        xqt = np.ascontiguousarray(
            x[b][:, qsl].T.astype(np.float16)
            .reshape(NJ, 128, C).transpose(1, 0, 2))
        in_maps.append({
            "xkv": np.ascontiguousarray(xkv),
            "xqt": xqt,
            "w1t": w1t,
            "w2t": w2t,
            "w3t": w3t,
        })
    return in_maps


def assemble(results):
    out = np.empty((B, C, N), dtype=np.float32)
    for core in range(8):
        b, half = core // 2, core % 2
        o = np.asarray(results[core]["out"], dtype=np.float32)  # [128, NJ, C]
        out[b][:, half * NQ:(half + 1) * NQ] = (
            o.transpose(1, 0, 2).reshape(NQ, C).T)
    return out.reshape(B, C, H, W)


def kernel(x, w1, w2, w3):
    nc = _get_nc()
    res = run_bass_kernel_spmd(nc, make_in_maps(x, w1, w2, w3),
                               core_ids=list(range(8)))
    return assemble(res.results)
